# revision 4
# baseline (speedup 1.0000x reference)
"""AELoss (associative-embedding push/pull loss) on 8 TRN2 NeuronCores.

Data-parallel over batch: each of the 8 cores handles 4 images. Only the
visible (person, joint) pairs are needed (~1020 per core).

v2 gather strategy: instead of C~8 serialized [128,1] SWDGE indirect DMAs
(994ns fixed descriptor-gen cost EACH on the Pool engine), issue TWO
InstDMAGatherAnt instructions (994ns fixed + 0.34ns/desc once per
instruction). Each gathers, for every visible joint of a 2-image half, the
512B tag-map row (128 f32) containing it; int16 row indices fit because
2*N/128 = 17408 < 32768. The within-row select happens on-chip against a
host-built one-hot mask.

Person-per-partition layout kills the per-column slot->person matmuls:
slot position s of a gather lands at (partition s%128, group s//128), and
the host assigns person q's first G visible joints to partition q
(gather A: persons 0..59 = images 0,1; gather B: persons 60..119). One
DVE tensor_tensor_reduce(raw, mask) per gather then yields per-PERSON
sum(g) directly in its accumulator, and one Scalar-engine Square
activation with accum_out yields sum(g^2) — no PE work for the segment
reduction at all. Joints beyond G per person ("foreign pieces", <=1 piece
per spare partition) are folded back with one tiny bf16 matmul pair.

Identities (exact vs the reference):
  pull_pp = sum(g^2 v)/safe_cnt - mean^2
  pull    = pull_num / max(n,1)
  push    = (S - n) * 0.5/max(n^2-n,1),  S via sqrt(pi)/2 * D_ERF
(Derivative_Erf(x) = 2/sqrt(pi) exp(-x^2); Square lives in the same
activation-table set, so one table load covers both.)
"""

import numpy as np

B, M, K = 32, 30, 17
N = 17 * 256 * 256
NCORES = 8
BL = B // NCORES          # images per core
P = 128
PERS = BL * M             # person rows per core (120)
ROWW = 128                # f32 per gathered row (512B)
HROWS = 2 * N // ROWW     # rows per 2-image half (17408 < int16 max)

SQPI2 = 0.8862269254527579   # sqrt(pi)/2: D_ERF(x) = 2/sqrt(pi) exp(-x^2)

# aux (f32) column layout
C_ONEHRC = 0              # [0,30)   oneh * rc
C_NEGRC = M               # 30       -1/max(cnt,1)
C_RC = M + 1              # 31       +1/max(cnt,1)
C_SELVI = M + 2           # [32,36)  sel*valid*inv_n
C_FINSUB = M + 2 + BL     # [36,38)  rows 0..3: [0, n*inv_nn]
C_RMA = C_FINSUB + 2      # 38       1.0 on rows 0..59  (gather A primary)
C_RMB = C_RMA + 1         # 39       1.0 on rows 60..119 (gather B primary)
W_AUX = C_RMB + 1         # 40

# bf16 tensor column layout: wimg [0,128) | rhs1 host half [128,158) |
# A2A foreign->person map [158,286) | A2B [286,414)
C_WIMG = 0
C_RHS1 = P
C_A2A = P + M
C_A2B = C_A2A + P
W_BF = C_A2B + P

_cache = {}


def _strip_init_barrier(nc):
    """Drop the Bass-init const-AP memsets and the all-engine barrier that
    orders them — nothing in this kernel reads the const APs (activation
    bias is passed as an explicit AP)."""
    import concourse.mybir as mybir

    bb = nc.main_func.blocks[0]
    drop = set()
    for ins in bb.instructions:
        if isinstance(ins, (mybir.InstMemset, mybir.InstDrain, mybir.InstEventSemaphore)):
            drop.add(ins.name)
    if not drop:
        return
    keep = [ins for ins in bb.instructions if ins.name not in drop]
    del bb.instructions[:]
    for ins in keep:
        bb.add_instruction(ins)


def _build(G):
    import concourse.bass as bass
    import concourse.bacc as bacc
    import concourse.mybir as mybir
    from concourse.tile import TileContext

    f32 = mybir.dt.float32
    bf16 = mybir.dt.bfloat16
    i16 = mybir.dt.int16
    X = mybir.AxisListType.X
    op = mybir.AluOpType
    act = mybir.ActivationFunctionType

    NI = P * G                # idxs per gather (all valid; holes hit row 0)
    IW = NI // 16             # idx int16 cols per gather (16-part wrap)
    MW = G * ROWW             # mask/raw cols per gather

    nc = bacc.Bacc(trn_type="TRN2", enable_partition_id=False,
                   num_swdge_queues=2)
    _strip_init_barrier(nc)
    tags_d = nc.dram_tensor("tags", [2 * HROWS, ROWW], f32, kind="ExternalInput")
    idx_d = nc.dram_tensor("idx16", [P, 2 * IW], i16, kind="ExternalInput")
    mask_d = nc.dram_tensor("mask", [P, 2 * MW], f32, kind="ExternalInput")
    aux_d = nc.dram_tensor("aux", [P, W_AUX], f32, kind="ExternalInput")
    bft_d = nc.dram_tensor("bft", [P, W_BF], bf16, kind="ExternalInput")
    out_d = nc.dram_tensor("out", [BL, 2], f32, kind="ExternalOutput")

    with TileContext(nc) as tc:
        with (
            tc.tile_pool(name="sb", bufs=1) as sb,
            tc.tile_pool(name="ps", bufs=1, space="PSUM") as ps,
        ):
            # idx16 first and alone on the scalar queue: the gathers' only
            # dependency, smallest possible transfer
            idx_t = sb.tile([P, 2 * IW], i16)
            nc.scalar.dma_start(out=idx_t[:], in_=idx_d[:])
            # masks ride sync (needed only when gather data lands, ~3us later)
            mask_t = sb.tile([P, 2 * MW], f32)
            nc.sync.dma_start(out=mask_t[:, 0:MW], in_=mask_d[:, 0:MW])
            nc.sync.dma_start(out=mask_t[:, MW:2 * MW], in_=mask_d[:, MW:2 * MW])
            # remaining small inputs on scalar
            aux_t = sb.tile([P, W_AUX], f32)
            nc.scalar.dma_start(out=aux_t[:], in_=aux_d[:])
            bft_t = sb.tile([P, W_BF], bf16)
            nc.scalar.dma_start(out=bft_t[:], in_=bft_d[:])

            # dummy D_ERF on DMA-ready data: pulls the erf_derivative ACT
            # table load (~1.3us, also covers Square) into the gather window
            dume = sb.tile([P, 1], f32)
            nc.scalar.activation(
                out=dume[:], in_=aux_t[:, C_RC:C_RC + 1],
                func=act.Derivative_Erf,
                bias=aux_t[:, C_NEGRC:C_NEGRC + 1], scale=1.0,
            )

            # two row-gathers: one SWDGE instruction each, separate queues so
            # the second's descriptor transfer overlaps the first's
            nreg = nc.gpsimd.to_reg(NI)
            rawA = sb.tile([P, MW], f32)
            rawB = sb.tile([P, MW], f32)
            nc.gpsimd.dma_gather(
                out_ap=rawA[:].rearrange("p (g w) -> p g w", w=ROWW),
                in_ap=tags_d[0:HROWS, :],
                idxs_ap=idx_t[:, 0:IW],
                num_idxs=NI, num_idxs_reg=nreg, elem_size=ROWW,
                queue_num=0,
            )
            nc.gpsimd.dma_gather(
                out_ap=rawB[:].rearrange("p (g w) -> p g w", w=ROWW),
                in_ap=tags_d[HROWS:2 * HROWS, :],
                idxs_ap=idx_t[:, IW:2 * IW],
                num_idxs=NI, num_idxs_reg=nreg, elem_size=ROWW,
                queue_num=1,
            )

            # per-partition sum(g) via one fused mult+reduce; masked tile is
            # the byproduct the Square pass consumes for sum(g^2)
            SA = sb.tile([P, 2], f32)
            SB = sb.tile([P, 2], f32)
            mskA = sb.tile([P, MW], f32)
            mskB = sb.tile([P, MW], f32)
            sqA = sb.tile([P, MW], f32)
            sqB = sb.tile([P, MW], f32)
            SA_bf = sb.tile([P, 2], bf16)
            SB_bf = sb.tile([P, 2], bf16)

            nc.vector.scalar_tensor_tensor(
                out=mskA[:], in0=rawA[:], scalar=1.0, in1=mask_t[:, 0:MW],
                op0=op.mult, op1=op.mult, accum_out=SA[:, 0:1],
            )
            nc.scalar.activation(
                out=sqA[:], in_=mskA[:], func=act.Square,
                accum_out=SA[:, 1:2],
            )
            nc.vector.scalar_tensor_tensor(
                out=mskB[:], in0=rawB[:], scalar=1.0, in1=mask_t[:, MW:2 * MW],
                op0=op.mult, op1=op.mult, accum_out=SB[:, 0:1],
            )
            nc.scalar.activation(
                out=sqB[:], in_=mskB[:], func=act.Square,
                accum_out=SB[:, 1:2],
            )

            # foreign pieces (joints beyond G per person) fold back via one
            # tiny bf16 matmul pair; A2 rows for primary partitions are zero,
            # so converting the whole S tile to bf16 is safe
            nc.vector.tensor_scalar(
                out=SA_bf[:], in0=SA[:], scalar1=1.0, scalar2=None, op0=op.mult,
            )
            nc.vector.tensor_scalar(
                out=SB_bf[:], in0=SB[:], scalar1=1.0, scalar2=None, op0=op.mult,
            )
            psF = ps.tile([P, 2], f32)
            nc.tensor.matmul(
                out=psF[:], lhsT=bft_t[:, C_A2A:C_A2A + P], rhs=SA_bf[:],
                start=True, stop=False, skip_group_check=True,
            )
            nc.tensor.matmul(
                out=psF[:], lhsT=bft_t[:, C_A2B:C_A2B + P], rhs=SB_bf[:],
                start=False, stop=True, skip_group_check=True,
            )

            # s12[p] = [sum g, sum g^2] of person p: primary rows of A and B
            # (disjoint row masks) plus the foreign-piece matmul result
            t1 = sb.tile([P, 2], f32)
            nc.vector.tensor_scalar(
                out=t1[:], in0=SA[:], scalar1=aux_t[:, C_RMA:C_RMA + 1],
                scalar2=None, op0=op.mult,
            )
            s12a = sb.tile([P, 2], f32)
            nc.vector.scalar_tensor_tensor(
                out=s12a[:], in0=SB[:], scalar=aux_t[:, C_RMB:C_RMB + 1],
                in1=t1[:], op0=op.mult, op1=op.add,
            )
            s12 = sb.tile([P, 2], f32)
            nc.vector.tensor_tensor(
                out=s12[:], in0=s12a[:], in1=psF[:], op=op.add,
            )

            # ---- tail (identical math to v1, reading s12) ----
            rhs1 = sb.tile([P, 2 * M], bf16)
            nc.sync.dma_start(out=rhs1[:, M:2 * M], in_=bft_d[:, C_RHS1:C_RHS1 + M])
            wimg_t = sb.tile([P, P], bf16)
            nc.scalar.dma_start(out=wimg_t[:], in_=bft_d[:, C_WIMG:C_WIMG + P])

            nc.vector.tensor_tensor(
                out=rhs1[:, 0:M], in0=aux_t[:, C_ONEHRC:C_ONEHRC + M],
                in1=s12[:, 0:1].to_broadcast([P, M]), op=op.mult,
            )
            negm = sb.tile([P, 1], f32)
            nc.vector.tensor_scalar(
                out=negm[:], in0=s12[:, 0:1],
                scalar1=aux_t[:, C_NEGRC:C_NEGRC + 1], scalar2=None, op0=op.mult,
            )
            msq = sb.tile([P, 1], f32)
            nc.vector.tensor_mul(out=msq[:], in0=negm[:], in1=negm[:])
            # rhs2 col0: pv = rc*sum_g2 - mean^2  (pull_pp, pre-valid)
            rhs2 = sb.tile([P, 2], f32)
            nc.vector.tensor_scalar(
                out=rhs2[:, 0:1], in0=s12[:, 1:2],
                scalar1=aux_t[:, C_RC:C_RC + 1], scalar2=msq[:],
                op0=op.mult, op1=op.subtract,
            )

            # same-image broadcast: m2[:,0:30]=means, m2[:,30:60]=mask*ratio
            m2 = ps.tile([P, 2 * M], f32)
            nc.tensor.matmul(
                out=m2[:], lhsT=wimg_t[:], rhs=rhs1[:], start=True, stop=True
            )

            # e = 2/sqrt(pi) * exp(-(mean_j - mean_p)^2), bias folds the sub
            e = sb.tile([P, M], f32)
            nc.scalar.activation(
                out=e[:], in_=m2[:, 0:M], func=act.Derivative_Erf,
                bias=negm[:, 0:1], scale=1.0,
            )
            # rhs2 col1: rowsum of masked e, fused mult+reduce
            em = sb.tile([P, M], f32)
            nc.vector.scalar_tensor_tensor(
                out=em[:], in0=e[:], scalar=1.0, in1=m2[:, M:2 * M],
                op0=op.mult, op1=op.mult, accum_out=rhs2[:, 1:2],
            )

            # fin[b] = [pull_b, push_b + n*inv_nn_b]
            fin = ps.tile([BL, 2], f32)
            nc.tensor.matmul(
                out=fin[:], lhsT=aux_t[:, C_SELVI:C_SELVI + BL], rhs=rhs2[:],
                start=True, stop=True,
            )
            outt = sb.tile([BL, 2], f32)
            nc.vector.tensor_sub(
                out=outt[:], in0=fin[:], in1=aux_t[0:BL, C_FINSUB:C_FINSUB + 2]
            )
            nc.sync.dma_start(out=out_d[:], in_=outt[:])

    nc.compile()
    return nc


def _pack_half(cnts, joints_flat, G):
    """Pack one 2-image half (60 persons) for a given primary window G.

    Returns (idx_rows[NI], mask[P, G*ROWW], foreign list of
    (partition, person_local, joint_flat_list)) or None if infeasible.
    Person q's first min(cnt, G) joints -> partition q groups 0..; leftover
    chunks of <=G go one-per-spare-partition.
    """
    pieces = []
    for q in range(60):
        jl = joints_flat[q]
        extra = jl[G:]
        while len(extra):
            pieces.append((q, extra[:G]))
            extra = extra[G:]
    return pieces


def _in_maps(tags, joints):
    import ml_dtypes

    tags = np.ascontiguousarray(np.asarray(tags, dtype=np.float32)).reshape(B, N)
    joints = np.asarray(joints, dtype=np.int32)
    idx_all = joints[..., 0]                               # [B, M, K]
    vis_all = joints[..., 1] > 0                           # [B, M, K] bool

    # balance images across the 16 two-image halves (LPT on visible count),
    # so the global primary window G stays minimal
    vis_cnt = vis_all.sum(axis=(1, 2))
    halves = [[] for _ in range(2 * NCORES)]
    hsum = [0] * (2 * NCORES)
    for i in np.argsort(-vis_cnt):
        h = min((x for x in range(2 * NCORES) if len(halves[x]) < 2),
                key=lambda x: hsum[x])
        halves[h].append(int(i))
        hsum[h] += int(vis_cnt[i])
    halves = [sorted(h) for h in halves]
    assign = [halves[2 * c] + halves[2 * c + 1] for c in range(NCORES)]

    # per-half person joint lists (flat element index within the half)
    half_joints = []                                       # [16][60] lists
    for h in range(2 * NCORES):
        pj = []
        for b, img in enumerate(halves[h]):
            for m in range(M):
                kk = np.nonzero(vis_all[img][m])[0]
                pj.append((idx_all[img][m, kk] + b * N).tolist())
        half_joints.append(pj)

    # minimal global G: primary window fits 60 persons' first G joints in
    # partitions 0..59 / 60..119; leftovers need <= 68 spare partitions
    G = 4
    while True:
        ok = True
        for pj in half_joints:
            pieces = _pack_half(None, pj, G)
            if len(pieces) > P - 60:
                ok = False
                break
        if ok:
            break
        G += 1

    NI = P * G
    IW = NI // 16
    MW = G * ROWW

    in_maps = []
    for c in range(NCORES):
        idx16 = np.zeros((P, 2 * IW), np.int16)
        mask = np.zeros((P, 2 * MW), np.float32)
        bft = np.zeros((P, W_BF), np.float32)
        aux = np.zeros((P, W_AUX), np.float32)
        aux[0:60, C_RMA] = 1.0
        aux[60:120, C_RMB] = 1.0

        for X in range(2):                                 # gather half
            pj = half_joints[2 * c + X]
            pbase = 60 * X                                 # primary partition base
            spare = [p for p in range(P) if not (pbase <= p < pbase + 60)]
            rows = np.zeros(NI, np.int64)                  # hole -> row 0
            si = 0                                         # spare cursor

            def place(part, grp, flat):
                s = grp * P + part
                rows[s] = flat // ROWW
                mask[part, X * MW + grp * ROWW + (flat % ROWW)] = 1.0

            for q in range(60):
                jl = pj[q]
                for g, flat in enumerate(jl[:G]):
                    place(pbase + q, g, flat)
            for q, piece in _pack_half(None, pj, G):
                part = spare[si]
                si += 1
                for g, flat in enumerate(piece):
                    place(part, g, flat)
                bft[part, (C_A2A if X == 0 else C_A2B) + pbase + q] = 1.0

            w = rows.reshape(IW, 16).T.astype(np.int16)    # [16, IW] wrap
            idx16[:, X * IW:(X + 1) * IW] = np.tile(w, (8, 1))

        # per-person aux / tail constants (person row = b*M + m)
        pp = np.arange(P)
        mrow = pp < PERS
        wimg = ((pp[:, None] // M == pp[None, :] // M)
                & mrow[:, None] & mrow[None, :]).astype(np.float32)
        bft[:, C_WIMG:C_WIMG + P] = wimg
        for b, img in enumerate(assign[c]):
            cnt = vis_all[img].sum(-1)                     # [M]
            valid = cnt > 0
            nb = int(valid.sum())
            inv_n = 1.0 / max(nb, 1)
            inv_nn = 0.5 / max(nb * nb - nb, 1)
            ratio = inv_nn / inv_n
            rows_sl = slice(b * M, (b + 1) * M)
            rcp = 1.0 / np.maximum(cnt, 1)
            aux[rows_sl, C_NEGRC] = -rcp
            aux[rows_sl, C_RC] = rcp
            aux[pp[rows_sl], C_ONEHRC + pp[rows_sl] % M] = rcp
            aux[rows_sl, C_SELVI + b] = valid * inv_n
            aux[b, C_FINSUB + 1] = nb * inv_nn
            bft[pp[rows_sl], C_RHS1 + pp[rows_sl] % M] = valid * (SQPI2 * ratio)

        in_maps.append({
            "tags": np.ascontiguousarray(
                tags[assign[c]].reshape(2 * HROWS, ROWW)),
            "idx16": idx16,
            "mask": mask,
            "aux": aux,
            "bft": bft.astype(ml_dtypes.bfloat16),
        })
    return G, in_maps, assign


def _run(key, in_maps, trace=False):
    from concourse import bass_utils

    if key not in _cache:
        _cache[key] = _build(key)
    return bass_utils.run_bass_kernel_spmd(
        _cache[key], in_maps, core_ids=list(range(NCORES)), trace=trace
    )


def kernel(tags, joints):
    key, in_maps, assign = _in_maps(tags, joints)
    res = _run(key, in_maps)
    push = np.zeros(B, np.float32)
    pull = np.zeros(B, np.float32)
    for c in range(NCORES):
        o = res.results[c]["out"]
        for b, img in enumerate(assign[c]):
            pull[img] = o[b, 0]
            push[img] = o[b, 1]
    return push, pull


# revision 12
# speedup vs baseline: 1.4326x; 1.4326x over previous
"""AELoss (associative-embedding push/pull loss) on 8 TRN2 NeuronCores.

Data-parallel over batch: each of the 8 cores handles 4 images. Only the
visible (person, joint) pairs are needed (~1020 per core); the host
compacts them into [128, C] slots (C ~ 8) and the kernel issues one
[128,1] SWDGE indirect gather per slot column (the ~1us/instruction
SWDGE fixed cost makes 128-offset columns the unit of gather work; the
hardware honors one offset per partition per instruction).

Per-person [sum g, sum g^2] accumulate in PSUM via one tiny PE matmul per
column against a host-built one-hot slot->person matrix, hidden under the
remaining gathers. The post-gather tail is minimized:
  - mean subtraction folds into the Derivative_Erf activation bias
  - the pair-mask multiply and row reduction fuse into one DVE
    tensor_tensor_reduce
  - the same-image broadcast matmul runs in bf16 (single PE pass)
  - all per-image scalars (1/n, 0.5/max(n^2-n,1), sqrt(pi)/2, the -n push
    offset) are host-folded into the matmul operands, leaving one
    tensor_sub before the output DMA:
      fin[b] = [pull_b, push_b + n*inv_nn_b]

Identities (exact vs the reference):
  pull_pp = sum(g^2 v)/safe_cnt - mean^2
  pull    = pull_num / max(n,1)
  push    = (S - n) * 0.5/max(n^2-n,1),  S via sqrt(pi)/2 * D_ERF
"""

import numpy as np

B, M, K = 32, 30, 17
N = 17 * 256 * 256
NCORES = 8
BL = B // NCORES          # images per core
P = 128
PERS = BL * M             # person rows per core (120)

SQPI2 = 0.8862269254527579   # sqrt(pi)/2: D_ERF(x) = 2/sqrt(pi) exp(-x^2)

# aux (f32) column layout
C_ONEHRC = 0              # [0,30)   oneh * rc
C_NEGRC = M               # 30       -1/max(cnt,1)
C_RC = M + 1              # 31       +1/max(cnt,1)
C_SELVI = M + 2           # [32,36)  sel*valid*inv_n
C_FINSUB = M + 2 + BL     # [36,38)  rows 0..3: [0, n*inv_nn]
W_AUX = C_FINSUB + 2      # 38

# bf16 tensor column layout: wimg [0,128) | rhs1 host half [128,158) |
# last-column amat block [158,158+JLMAX) (bf16 single-pass PE for the
# tail-gating column)
C_AML = P + M
JLMAX = P
W_BF = C_AML + JLMAX

_cache = {}


def _strip_init_barrier(nc):
    """Drop the Bass-init const-AP memsets and the all-engine barrier that
    orders them — nothing in this kernel reads the const APs (activation
    bias is passed as an explicit AP)."""
    import concourse.mybir as mybir

    bb = nc.main_func.blocks[0]
    drop = set()
    for ins in bb.instructions:
        if isinstance(ins, (mybir.InstMemset, mybir.InstDrain, mybir.InstEventSemaphore)):
            drop.add(ins.name)
    if not drop:
        return
    keep = [ins for ins in bb.instructions if ins.name not in drop]
    del bb.instructions[:]
    for ins in keep:
        bb.add_instruction(ins)


def _build(C, JL):
    import concourse.bass as bass
    import concourse.bacc as bacc
    import concourse.mybir as mybir
    from concourse.tile import TileContext

    f32 = mybir.dt.float32
    bf16 = mybir.dt.bfloat16
    i32 = mybir.dt.int32
    X = mybir.AxisListType.X
    op = mybir.AluOpType

    # no partition-id loads: SPMD per-core data arrives via in_maps, nothing
    # branches on core id, and the 5 per-engine ~1us DRAM register loads the
    # id tensor costs sit right in the startup critical path
    nc = bacc.Bacc(trn_type="TRN2", enable_partition_id=False)
    _strip_init_barrier(nc)
    tags_d = nc.dram_tensor("tags", [BL * N, 1], f32, kind="ExternalInput")
    idx_d = nc.dram_tensor("idx", [P, C], i32, kind="ExternalInput")
    amat_d = nc.dram_tensor("amat", [P, C * P], f32, kind="ExternalInput")
    aux_d = nc.dram_tensor("aux", [P, W_AUX], f32, kind="ExternalInput")
    bft_d = nc.dram_tensor("bft", [P, W_BF], bf16, kind="ExternalInput")
    out_d = nc.dram_tensor("out", [BL, 2], f32, kind="ExternalOutput")

    with TileContext(nc) as tc:
        with (
            tc.tile_pool(name="sb", bufs=1) as sb,
            tc.tile_pool(name="ps", bufs=1, space="PSUM") as ps,
        ):
            # idx first: column 0 rides its own minimal DMA so the first
            # gather's wait is a 4B/partition transfer; scalar's queue opens
            # earlier than sync's (whose first dispatch sits behind a long
            # drain), so idx0 goes there and the rest rides sync
            idx0_t = sb.tile([P, 1], i32)
            nc.scalar.dma_start(out=idx0_t[:], in_=idx_d[:, 0:1])
            idxr_t = sb.tile([P, C - 1], i32)
            nc.sync.dma_start(out=idxr_t[:], in_=idx_d[:, 1:C])

            # gathers: one [128,1] indirect DMA per packed slot column;
            # bounds register hoisted so each gather skips its own MOVE.
            # (Tried one tile per column to break the apparent inter-gather
            # ordering — measured ~1.7us SLOWER; the shared tile stays.)
            br = nc.gpsimd.to_reg(BL * N - 1)
            gg = sb.tile([P, 2 * C], f32)
            for c in range(C):
                off = idx0_t[:, 0:1] if c == 0 else idxr_t[:, c - 1:c]
                nc.gpsimd.indirect_dma_start(
                    out=gg[:, 2 * c:2 * c + 1],
                    out_offset=None,
                    in_=tags_d[:],
                    in_offset=bass.IndirectOffsetOnAxis(ap=off, axis=0),
                    oob_is_err=False,
                    bounds_check=br,
                )

            # remaining small inputs on the scalar engine's HWDGE queue
            amat_t = sb.tile([P, C * P], f32)
            nc.scalar.dma_start(out=amat_t[:], in_=amat_d[:])
            aux_t = sb.tile([P, W_AUX], f32)
            nc.scalar.dma_start(out=aux_t[:], in_=aux_d[:])
            wimg_t = sb.tile([P, P], bf16)
            nc.scalar.dma_start(out=wimg_t[:], in_=bft_d[:, 0:P])
            rhs1 = sb.tile([P, 2 * M], bf16)
            nc.sync.dma_start(out=rhs1[:, M:2 * M], in_=bft_d[:, P:P + M])
            amlt = sb.tile([P, JL], bf16)
            nc.sync.dma_start(out=amlt[:], in_=bft_d[:, C_AML:C_AML + JL])

            # dummy D_ERF on DMA-ready data: pulls the ACT table load into
            # the gather window (the real activation's first dep is the
            # engine-computed bias, which would otherwise drag the ~1.3us
            # table load onto the critical path)
            dume = sb.tile([P, 1], f32)
            nc.scalar.activation(
                out=dume[:], in_=aux_t[:, C_RC:C_RC + 1],
                func=mybir.ActivationFunctionType.Derivative_Erf,
                bias=aux_t[:, C_NEGRC:C_NEGRC + 1], scale=1.0,
            )

            # per-person [sum g, sum g^2] via one accumulating matmul per
            # column: square each landed column, matmul against the one-hot
            # slot->person matrix (overlaps the remaining gathers). The LAST
            # column is the tail gate: run it in bf16 (one PE pass instead of
            # the fp32 two-pass) against a host-provided bf16 amat block —
            # only the ~124 lowest-person slots lose ~0.4% there.
            ggl_bf = sb.tile([P, 2], bf16)
            ps12 = ps.tile([P, 2], f32)
            for c in range(C):
                if c == C - 1:
                    # bf16 (g, g^2) pair for the last column: two tiny DVE
                    # writes, then a single-pass bf16 matmul
                    nc.vector.tensor_scalar(
                        out=ggl_bf[:, 0:1], in0=gg[:, 2 * c:2 * c + 1],
                        scalar1=1.0, scalar2=None, op0=op.mult,
                    )
                    nc.vector.tensor_mul(
                        out=ggl_bf[:, 1:2],
                        in0=gg[:, 2 * c:2 * c + 1], in1=gg[:, 2 * c:2 * c + 1],
                    )
                    nc.tensor.matmul(
                        out=ps12[:JL, :], lhsT=amlt[:, 0:JL],
                        rhs=ggl_bf[:], start=False, stop=True,
                        skip_group_check=True,
                    )
                else:
                    nc.vector.tensor_mul(
                        out=gg[:, 2 * c + 1:2 * c + 2],
                        in0=gg[:, 2 * c:2 * c + 1], in1=gg[:, 2 * c:2 * c + 1],
                    )
                    nc.tensor.matmul(
                        out=ps12[:, :], lhsT=amat_t[:, c * P:c * P + P],
                        rhs=gg[:, 2 * c:2 * c + 2],
                        start=(c == 0), stop=False,
                        skip_group_check=True,
                    )

            # ---- post-gather tail (reads the PSUM sums directly) ----
            # rhs1 cols 0:30 = oneh*rc*sum_g (bf16 for the broadcast matmul)
            nc.vector.tensor_tensor(
                out=rhs1[:, 0:M], in0=aux_t[:, C_ONEHRC:C_ONEHRC + M],
                in1=ps12[:, 0:1].to_broadcast([P, M]), op=op.mult,
            )
            negm = sb.tile([P, 1], f32)
            nc.vector.tensor_scalar(
                out=negm[:], in0=ps12[:, 0:1],
                scalar1=aux_t[:, C_NEGRC:C_NEGRC + 1], scalar2=None, op0=op.mult,
            )
            # msq = (rc*sum_g)^2 on the otherwise-idle ACT engine (scale
            # folds the rc mult into the Square), parallel to the DVE chain
            msq = sb.tile([P, 1], f32)
            nc.scalar.activation(
                out=msq[:], in_=ps12[:, 0:1],
                func=mybir.ActivationFunctionType.Square,
                scale=aux_t[:, C_NEGRC:C_NEGRC + 1],
            )
            # rhs2 col0: pv = rc*sum_g2 - mean^2  (pull_pp, pre-valid)
            rhs2 = sb.tile([P, 2], f32)
            nc.vector.tensor_scalar(
                out=rhs2[:, 0:1], in0=ps12[:, 1:2],
                scalar1=aux_t[:, C_RC:C_RC + 1], scalar2=msq[:],
                op0=op.mult, op1=op.subtract,
            )

            # same-image broadcast: m2[:,0:30]=means, m2[:,30:60]=mask*ratio
            m2 = ps.tile([P, 2 * M], f32)
            nc.tensor.matmul(
                out=m2[:], lhsT=wimg_t[:], rhs=rhs1[:], start=True, stop=True
            )

            # e = 2/sqrt(pi) * exp(-(mean_j - mean_p)^2), bias folds the sub
            e = sb.tile([P, M], f32)
            nc.scalar.activation(
                out=e[:], in_=m2[:, 0:M],
                func=mybir.ActivationFunctionType.Derivative_Erf,
                bias=negm[:, 0:1], scale=1.0,
            )
            # rhs2 col1: rowsum of masked e (mask carries valid_j*ratio*sqpi2),
            # fused mult+reduce in one DVE pass
            em = sb.tile([P, M], f32)
            nc.vector.scalar_tensor_tensor(
                out=em[:], in0=e[:], scalar=1.0, in1=m2[:, M:2 * M],
                op0=op.mult, op1=op.mult, accum_out=rhs2[:, 1:2],
            )

            # fin[b] = [pull_b, push_b + n*inv_nn_b]
            fin = ps.tile([BL, 2], f32)
            nc.tensor.matmul(
                out=fin[:], lhsT=aux_t[:, C_SELVI:C_SELVI + BL], rhs=rhs2[:],
                start=True, stop=True,
            )
            outt = sb.tile([BL, 2], f32)
            nc.vector.tensor_sub(
                out=outt[:], in0=fin[:], in1=aux_t[0:BL, C_FINSUB:C_FINSUB + 2]
            )
            nc.sync.dma_start(out=out_d[:], in_=outt[:])

    nc.compile()
    return nc


def _in_maps(tags, joints):
    import ml_dtypes

    tags = np.ascontiguousarray(np.asarray(tags, dtype=np.float32)).reshape(B, N)
    joints = np.asarray(joints, dtype=np.int32)
    idx_all = joints[..., 0]                               # [B, M, K]
    vis_all = joints[..., 1] > 0                           # [B, M, K] bool

    # balance images across cores so the max per-core visible-slot count
    # (which sets the gather-column count C) is minimized: LPT bin packing
    vis_cnt = vis_all.sum(axis=(1, 2))
    bins = [[] for _ in range(NCORES)]
    sums = [0] * NCORES
    for i in np.argsort(-vis_cnt):
        c = min((b for b in range(NCORES) if len(bins[b]) < BL),
                key=lambda b: sums[b])
        bins[c].append(int(i))
        sums[c] += int(vis_cnt[i])
    assign = [sorted(b) for b in bins]                     # [NCORES][BL] image ids

    # compact visible (person, joint) slots per core
    per_core = []
    C = 1
    for c in range(NCORES):
        persons = []
        fidx = []
        for b, img in enumerate(assign[c]):
            vb = vis_all[img]                              # [M, K]
            mm, kk = np.nonzero(vb)
            persons.append(b * M + mm)
            fidx.append(idx_all[img][mm, kk] + b * N)
        persons = np.concatenate(persons)
        fidx = np.concatenate(fidx)
        order = np.argsort(-persons, kind="stable")
        persons, fidx = persons[order], fidx[order]
        per_core.append((persons, fidx))
        C = max(C, (len(fidx) + P - 1) // P)

    # slots are person-descending, so the last column only involves the
    # lowest person ids: its A matmul can use a narrow LDWEIGHTS
    JL = 1
    lo = (C - 1) * P
    for persons, fidx in per_core:
        if len(fidx) > lo:
            JL = max(JL, int(persons[lo:].max()) + 1)
    JL = min(P, ((JL + 15) // 16) * 16)

    pp = np.arange(P)
    mrow = pp < PERS
    wimg = ((pp[:, None] // M == pp[None, :] // M)
            & mrow[:, None] & mrow[None, :]).astype(np.float32)

    in_maps = []
    for c in range(NCORES):
        persons, fidx = per_core[c]
        n_slots = len(fidx)
        idx_l = np.zeros((P, C), np.int32)
        amat = np.zeros((P, C * P), np.float32)
        s = np.arange(n_slots)
        sp, scol = s % P, s // P
        idx_l[sp, scol] = fidx
        amat[sp, scol * P + persons] = 1.0

        aux = np.zeros((P, W_AUX), np.float32)
        bft = np.zeros((P, W_BF), np.float32)
        bft[:, 0:P] = wimg
        # bf16 copy of the last amat column block (the tail-gating matmul)
        bft[:, C_AML:C_AML + JL] = amat[:, (C - 1) * P:(C - 1) * P + JL]
        for b, img in enumerate(assign[c]):
            cnt = vis_all[img].sum(-1)                     # [M]
            valid = cnt > 0
            nb = int(valid.sum())
            inv_n = 1.0 / max(nb, 1)
            inv_nn = 0.5 / max(nb * nb - nb, 1)
            ratio = inv_nn / inv_n
            rows = slice(b * M, (b + 1) * M)
            rcp = 1.0 / np.maximum(cnt, 1)
            aux[rows, C_NEGRC] = -rcp
            aux[rows, C_RC] = rcp
            aux[pp[rows], C_ONEHRC + pp[rows] % M] = rcp
            aux[rows, C_SELVI + b] = valid * inv_n
            aux[b, C_FINSUB + 1] = nb * inv_nn
            bft[pp[rows], P + pp[rows] % M] = valid * (SQPI2 * ratio)
        in_maps.append({
            "tags": np.ascontiguousarray(tags[assign[c]].reshape(BL * N, 1)),
            "idx": idx_l,
            "amat": amat,
            "aux": aux,
            "bft": bft.astype(ml_dtypes.bfloat16),
        })
    return (C, JL), in_maps, assign


def _run(key, in_maps, trace=False):
    from concourse import bass_utils

    if key not in _cache:
        _cache[key] = _build(*key)
    return bass_utils.run_bass_kernel_spmd(
        _cache[key], in_maps, core_ids=list(range(NCORES)), trace=trace
    )


def kernel(tags, joints):
    key, in_maps, assign = _in_maps(tags, joints)
    res = _run(key, in_maps)
    push = np.zeros(B, np.float32)
    pull = np.zeros(B, np.float32)
    for c in range(NCORES):
        o = res.results[c]["out"]
        for b, img in enumerate(assign[c]):
            pull[img] = o[b, 0]
            push[img] = o[b, 1]
    return push, pull

